# revision 1
# baseline (speedup 1.0000x reference)
"""Trainium2 Bass kernel for nn_Attention (LN -> QKV -> RoPE -> softmax attn -> out-proj).

Sharding: tensor-parallel over heads. Each of the 8 cores computes 2 of the 16
heads for both batches (column-split w_qkv, row-split w_out) and produces a
partial (DIM, B*N) output in transposed layout; the host sums the 8 partials.

Device-side layout is activation-transposed: x^T [DIM, TOK] streams through a
bf16 QKV matmul producing q/k/v as [cols, tokens]; LayerNorm is folded into the
weights (W' = diag(ln_w) @ W) plus a rank-1 per-token correction
(qkv = s_t * x@W' + b_t * u + v) whose per-token scalars come from ones-column
matmul statistics. RoPE is a partition pair-swap (stream_shuffle) + cos/sin
tables. Attention runs in the S^T orientation (scores [keys, queries]) so the
P@V matmul needs no transposes; softmax denominators ride the attnV matmul as
an appended ones-column of V. All attention matmuls use float32r.
"""
import sys
sys.path.insert(0, "/opt/trn_rl_repo")

import numpy as np
import ml_dtypes
from contextlib import ExitStack

import bass_rust
import concourse.bass as bass
import concourse.tile as tile
from concourse import mybir

F32 = mybir.dt.float32
F32R = mybir.dt.float32r
BF16 = mybir.dt.bfloat16
AF = mybir.ActivationFunctionType
OP = mybir.AluOpType

# ---------------------------------------------------------------------------
# walrus in this image rejects >1 sync-wait on a Drain (CTRL) instruction;
# split the TileContext epilogue drain into a chain of single-wait drains.
_orig_drain_and_barrier = tile.TileContext._drain_and_barrier


def _split_drain_and_barrier(self, tick_clock, wait_clock):
    from bass_rust import ScopedClock

    drain_inst = self.nc.sync.drain()
    wait_clock.add_sem_waits(drain_inst.ins, ScopedClock({None: tick_clock.global_clock}))
    waits = list(drain_inst.ins.sync_info.on_wait)
    if len(waits) > 1:
        ups = list(drain_inst.ins.sync_info.on_update)
        drain_inst.ins.sync_info = bass_rust.SyncInfo(on_wait=waits[:1], on_update=[])
        rest = waits[1:]
        while rest:
            chunk, rest = rest[:1], rest[1:]
            d2 = self.nc.sync.drain()
            d2.ins.sync_info = bass_rust.SyncInfo(
                on_wait=chunk, on_update=[] if rest else ups
            )
    self.nc.all_engine_barrier()
    assert self.sems is not None
    popped = self.nc._tile_sem_poison_stack.pop()
    assert popped is self._sem_poison
    self.nc.clear_and_free_semaphores(list(self.sems.allocated().values()))
    self.nc.all_engine_barrier()


tile.TileContext._drain_and_barrier = _split_drain_and_barrier

_WAIT_CAP = 1


def split_excess_waits(nc):
    """walrus in this image caps sync-waits per instruction very low. Move
    excess waits onto same-engine NOPs inserted immediately before the
    instruction (engine queues are in-order, so the gating is preserved)."""
    nid = [0]

    def mk_nop(engine, waits):
        nid[0] += 1
        n = bass_rust.InstNoOp(name=f"WSPL-{nid[0]}", engine=engine, ins=[], outs=[])
        n.sync_info = bass_rust.SyncInfo(on_wait=waits, on_update=[])
        return n

    for f in nc.m.functions:
        for bb in f.blocks:
            out = []
            for inst in bb.instructions:
                si = inst.sync_info
                waits = list(si.on_wait) if si is not None else []
                if len(waits) > _WAIT_CAP:
                    keep = waits[: _WAIT_CAP]
                    rest = waits[_WAIT_CAP:]
                    while rest:
                        chunk, rest = rest[:_WAIT_CAP], rest[_WAIT_CAP:]
                        out.append(mk_nop(inst.engine, chunk))
                    inst.sync_info = bass_rust.SyncInfo(
                        on_wait=keep, on_update=list(si.on_update))
                out.append(inst)
            bb.instructions = out


# ---------------------------------------------------------------------------
class Cfg:
    def __init__(self, DIM=1024, NB=2, NPB=2048, DH=64, H=2, IC=512, eps=1e-5):
        self.DIM, self.NB, self.NPB, self.DH, self.H = DIM, NB, NPB, DH, H
        self.TOK = NB * NPB
        self.KC = DIM // 128          # k-chunks of the QKV contraction
        self.QC = H * DH              # q/k/v columns per core (128)
        self.FC = 512                 # free chunk for QKV/outproj/stats
        self.TFC = self.TOK // self.FC
        self.JT = NPB // 128          # key tiles per batch
        self.IC = IC                  # query chunk
        self.ICN = NPB // IC
        self.DO = DIM                 # out-proj output dim
        self.DOT = DIM // 128
        self.eps = eps
        assert self.QC == 128 and DIM % 128 == 0 and NPB % 128 == 0
        assert self.TOK % self.FC == 0 and NPB % IC == 0


def build_nc(c: Cfg, split_waits: bool = True):
    nc = bass.Bass("TRN2", target_bir_lowering=False)

    xt_d = nc.dram_tensor("xt", [128, c.KC, c.TOK], BF16, kind="ExternalInput")
    wq_d = nc.dram_tensor("wq", [128, c.KC, 3 * c.QC], BF16, kind="ExternalInput")
    fixc_d = nc.dram_tensor("fixc", [128, 6], F32, kind="ExternalInput")
    cosk_d = nc.dram_tensor("cosk", [128, c.NPB], F32, kind="ExternalInput")
    sink_d = nc.dram_tensor("sink", [128, c.NPB], F32, kind="ExternalInput")
    wout_d = nc.dram_tensor("wout", [128, c.DO], F32R, kind="ExternalInput")
    bout_d = nc.dram_tensor("bout", [128, c.DOT], F32, kind="ExternalInput")
    selstat_d = nc.dram_tensor("selstat", [c.TFC, c.TFC * 128], F32, kind="ExternalInput")
    selden_d = nc.dram_tensor("selden", [2 * c.ICN, c.ICN * 128], F32R, kind="ExternalInput")
    idblk_d = nc.dram_tensor("idblk", [128, 64], F32, kind="ExternalInput")
    out_d = nc.dram_tensor("out", [128, c.DOT, c.TOK], F32, kind="ExternalOutput")

    def r(ap):
        return ap.bitcast(F32R)

    with ExitStack() as ctx:
        tc = ctx.enter_context(tile.TileContext(nc))
        wp = ctx.enter_context(tc.tile_pool(name="wp", bufs=1))
        big = ctx.enter_context(tc.tile_pool(name="big", bufs=1))

        # --- constants / weights ---
        wq = wp.tile([128, c.KC, 3 * c.QC], BF16)
        nc.sync.dma_start(wq[:], wq_d[:])
        fixc = wp.tile([128, 6], F32)
        nc.sync.dma_start(fixc[:], fixc_d[:])
        wout = wp.tile([128, c.DO], F32R)
        nc.sync.dma_start(wout[:], wout_d[:])
        bout = wp.tile([128, c.DOT], F32)
        nc.sync.dma_start(bout[:], bout_d[:])
        nc.vector.tensor_scalar_mul(bout[:], bout[:], 0.125)  # each core adds b/8
        selstat = wp.tile([c.TFC, c.TFC * 128], F32)
        nc.sync.dma_start(selstat[:], selstat_d[:])
        selden = wp.tile([2 * c.ICN, c.ICN * 128], F32R)
        nc.sync.dma_start(selden[:], selden_d[:])
        tabs = {}
        for nm, d in (("cosk", cosk_d), ("sink", sink_d)):
            t = wp.tile([128, c.NPB], F32, name=nm)
            nc.sync.dma_start(t[:], d[:])
            tabs[nm] = t
        idblk = wp.tile([128, 64], F32)
        nc.sync.dma_start(idblk[:], idblk_d[:])
        ones_bf = wp.tile([128, 1], BF16)
        nc.vector.memset(ones_bf[:], 1.0)
        ones_f = wp.tile([128, 1], F32)
        nc.vector.memset(ones_f[:], 1.0)

        # persistent activations
        qraw = big.tile([128, c.TOK], F32R)
        kraw = big.tile([128, c.TOK], F32R)
        vraw = big.tile([128, c.TOK], F32)

        # ---------------- phase A: stats + QKV ----------------
        with ExitStack() as actx:
            sbp = actx.enter_context(tc.tile_pool(name="sbp", bufs=1))
            s_b = sbp.tile([128, c.TOK], F32)   # broadcast rsqrt(var+eps)
            b_b = sbp.tile([128, c.TOK], F32)   # broadcast -mu*s
            xp = actx.enter_context(tc.tile_pool(name="xp", bufs=2))
            sqp = actx.enter_context(tc.tile_pool(name="sqp", bufs=3))
            stp = actx.enter_context(tc.tile_pool(name="stp", bufs=1))
            stps = actx.enter_context(tc.tile_pool(name="stps", bufs=2, space="PSUM"))
            bcps = actx.enter_context(tc.tile_pool(name="bcps", bufs=2, space="PSUM"))
            qps = actx.enter_context(tc.tile_pool(name="qps", bufs=3, space="PSUM"))

            NQ = max(1, c.TFC // 2)          # token quarters
            QTOK = c.TOK // NQ
            QFC = QTOK // c.FC
            xts = []
            for tq in range(NQ):
                xt = xp.tile([128, c.KC, QTOK], BF16, tag="xt", name="xt")
                nc.sync.dma_start(xt[:], xt_d[:, :, tq * QTOK:(tq + 1) * QTOK])
                xts.append(xt)

            s1pk = stp.tile([c.TFC, c.FC], F32)
            s2pk = stp.tile([c.TFC, c.FC], F32)
            st_stage = stp.tile([1, 2, c.TFC, c.FC], F32)
            for fc in range(c.TFC):
                xt = xts[fc // QFC]
                fsl = slice((fc % QFC) * c.FC, (fc % QFC + 1) * c.FC)
                s1p = stps.tile([1, c.FC], F32, tag="st")
                for kc in range(c.KC):
                    nc.tensor.matmul(s1p[:], ones_bf[:], xt[:, kc, fsl],
                                     start=(kc == 0), stop=(kc == c.KC - 1))
                nc.scalar.copy(st_stage[0:1, 0, fc, :], s1p[:])
                s2p = stps.tile([1, c.FC], F32, tag="st")
                for kc in range(c.KC):
                    sq = sqp.tile([128, c.FC], BF16, tag="sq")
                    nc.vector.tensor_tensor(sq[:], xt[:, kc, fsl], xt[:, kc, fsl], OP.mult)
                    nc.tensor.matmul(s2p[:], ones_bf[:], sq[:],
                                     start=(kc == 0), stop=(kc == c.KC - 1))
                nc.scalar.copy(st_stage[0:1, 1, fc, :], s2p[:])

            nc.sync.dma_start(s1pk[:], st_stage[0:1, 0])
            nc.sync.dma_start(s2pk[:], st_stage[0:1, 1])
            # per-token scalars: s = rsqrt(var+eps), b = -mu*s  (packed [TFC, FC])
            mun = stp.tile([c.TFC, c.FC], F32)
            nc.vector.tensor_scalar_mul(mun[:], s1pk[:], -1.0 / c.DIM)
            nc.vector.tensor_scalar_mul(s2pk[:], s2pk[:], 1.0 / c.DIM)
            mu2 = stp.tile([c.TFC, c.FC], F32)
            nc.vector.tensor_tensor(mu2[:], mun[:], mun[:], OP.mult)
            nc.vector.tensor_tensor(s2pk[:], s2pk[:], mu2[:], OP.subtract)
            nc.vector.tensor_scalar_add(s2pk[:], s2pk[:], c.eps)
            rv = stp.tile([c.TFC, c.FC], F32)
            nc.vector.reciprocal(rv[:], s2pk[:])
            spk = stp.tile([c.TFC, c.FC], F32)
            nc.scalar.sqrt(spk[:], rv[:])
            bpk = stp.tile([c.TFC, c.FC], F32)
            nc.vector.tensor_tensor(bpk[:], mun[:], spk[:], OP.mult)

            # broadcast s,b across partitions via one-hot selector matmuls
            for fc in range(c.TFC):
                fsl = slice(fc * c.FC, (fc + 1) * c.FC)
                sel = selstat[:, fc * 128:(fc + 1) * 128]
                for src, dst in ((spk, s_b), (bpk, b_b)):
                    bp = bcps.tile([128, c.FC], F32, tag="bc")
                    nc.tensor.matmul(bp[:], sel, src[:], start=True, stop=True)
                    nc.vector.tensor_copy(dst[:, fsl], bp[:])

            # QKV matmuls (bf16) -> raw q/k/v; LN fixup applied after stats
            for fc in range(c.TFC):
                fsl = slice(fc * c.FC, (fc + 1) * c.FC)
                xt = xts[fc // QFC]
                xsl = slice((fc % QFC) * c.FC, (fc % QFC + 1) * c.FC)
                for m, dst in enumerate((qraw, kraw, vraw)):
                    qp = qps.tile([128, c.FC], F32, tag="q")
                    for kc in range(c.KC):
                        nc.tensor.matmul(qp[:], wq[:, kc, m * 128:(m + 1) * 128],
                                         xt[:, kc, xsl],
                                         start=(kc == 0), stop=(kc == c.KC - 1))
                    nc.scalar.copy(dst[:, fsl], qp[:])
            # fixup pass: dst = dst * s_b + (b_b * u + v)
            for fc in range(c.TFC):
                fsl = slice(fc * c.FC, (fc + 1) * c.FC)
                for m, dst in enumerate((qraw, kraw, vraw)):
                    tmp = sqp.tile([128, c.FC], F32, tag="fx")
                    nc.vector.tensor_scalar(tmp[:], b_b[:, fsl],
                                            fixc[:, m:m + 1], fixc[:, 3 + m:4 + m],
                                            OP.mult, OP.add)
                    nc.vector.tensor_tensor(dst[:, fsl], dst[:, fsl], s_b[:, fsl], OP.mult)
                    nc.vector.tensor_tensor(dst[:, fsl], dst[:, fsl], tmp[:], OP.add)

        # ---------------- phase A2: RoPE + V layout ----------------
        bp2 = ctx.enter_context(tc.tile_pool(name="bp2", bufs=1))
        qbf = bp2.tile([128, c.TOK], BF16)
        kbf = bp2.tile([128, c.TOK], BF16)
        vaug = bp2.tile([128, c.NB, c.H, c.JT, c.DH + 1], BF16)
        ctx_sb = bp2.tile([128, c.NB, c.NPB], F32R)
        dens = bp2.tile([2 * c.ICN, c.NB, c.IC], F32R)
        dens_stage = bp2.tile([1, c.NB, 2 * c.ICN, c.IC], F32R)
        pairswap = [i ^ 1 for i in range(32)]
        with ExitStack() as a2ctx:
            shp = a2ctx.enter_context(tc.tile_pool(name="shp", bufs=1))
            vps = a2ctx.enter_context(tc.tile_pool(name="vps", bufs=3, space="PSUM"))
            for hb in range(c.NB):
                hsl = slice(hb * c.NPB, (hb + 1) * c.NPB)
                for src, cn, sn in ((qraw, "cosk", "sink"), (kraw, "cosk", "sink")):
                    sh = shp.tile([128, c.NPB], F32, tag="sh")
                    nc.vector.stream_shuffle(sh[:], src[:, hsl].bitcast(F32), pairswap)
                    nc.vector.tensor_tensor(src[:, hsl], src[:, hsl], tabs[cn][:], OP.mult)
                    nc.vector.tensor_tensor(sh[:], sh[:], tabs[sn][:], OP.mult)
                    nc.vector.tensor_tensor(src[:, hsl], src[:, hsl], sh[:], OP.add)
                    dstbf = qbf if src is qraw else kbf
                    nc.vector.tensor_copy(dstbf[:, hsl], src[:, hsl].bitcast(F32))

            nc.scalar.copy(vaug[:, :, :, :, c.DH],
                           ones_bf[:, 0:1].to_broadcast([128, c.NB, c.H, c.JT]))
            for hb in range(c.NB):
                for hh in range(c.H):
                    for jt in range(c.JT):
                        vin = vraw[hh * c.DH:(hh + 1) * c.DH,
                                   hb * c.NPB + jt * 128: hb * c.NPB + (jt + 1) * 128]
                        tp = vps.tile([128, c.DH], F32, tag="vt")
                        dsl = slice(hh * c.DH, (hh + 1) * c.DH)
                        nc.tensor.transpose(tp[:], vin, idblk[dsl, :])
                        nc.scalar.copy(vaug[:, hb, hh, jt, 0:c.DH], tp[:])

        # ---------------- phase B/C: attention + out-proj ----------------
        with ExitStack() as bctx:
            ep = bctx.enter_context(tc.tile_pool(name="ep", bufs=3))
            ysb = bctx.enter_context(tc.tile_pool(name="ysb", bufs=4))
            sps = bctx.enter_context(tc.tile_pool(name="sps", bufs=3, space="PSUM"))
            cps = bctx.enter_context(tc.tile_pool(name="cps", bufs=1, space="PSUM"))
            yps = sps

            for hb in range(c.NB):
                hoff = hb * c.NPB
                for ic in range(c.ICN):
                    isl = slice(hoff + ic * c.IC, hoff + (ic + 1) * c.IC)
                    cpss = [cps.tile([c.DH + 1, c.IC], F32, tag=f"ctx{hh}", name=f"ctx{hh}")
                            for hh in range(c.H)]
                    for jt in range(c.JT):
                        sp = sps.tile([128, 2 * c.IC], F32, tag="sp", name="sp")
                        for hh in range(c.H):
                            dsl = slice(hh * c.DH, (hh + 1) * c.DH)
                            nc.tensor.matmul(
                                sp[:, hh * c.IC:(hh + 1) * c.IC],
                                kbf[dsl, hoff + jt * 128: hoff + (jt + 1) * 128],
                                qbf[dsl, isl],
                                start=True, stop=True,
                                tile_position=(hh * c.DH, 0))
                        e = ep.tile([128, 2 * c.IC], BF16, tag="e", name="e")
                        nc.scalar.activation(e[:], sp[:], AF.Exp)
                        for hh in range(c.H):
                            nc.tensor.matmul(
                                cpss[hh][:],
                                vaug[:, hb, hh, jt, :],
                                e[:, hh * c.IC:(hh + 1) * c.IC],
                                start=(jt == 0), stop=(jt == c.JT - 1))
                    for hh in range(c.H):
                        nc.scalar.copy(dens_stage[0:1, hb, hh * c.ICN + ic, :],
                                       cpss[hh][c.DH:c.DH + 1, :])
                        nc.scalar.copy(ctx_sb[hh * c.DH:(hh + 1) * c.DH, hb,
                                              ic * c.IC:(ic + 1) * c.IC],
                                       cpss[hh][0:c.DH, :])
                # normalize + out-proj per query chunk
                nc.sync.dma_start(dens[:, hb, :], dens_stage[0:1, hb])
                with nc.allow_low_precision(reason="softmax denom reciprocal in fp32r"):
                    nc.vector.reciprocal(dens[:, hb, :], dens[:, hb, :])
                for ic in range(c.ICN):
                    csl = slice(ic * c.IC, (ic + 1) * c.IC)
                    rp = sps.tile([128, 2 * c.IC], F32, tag="sp", name="rp")
                    nc.tensor.matmul(rp[:, :c.IC],
                                     selden[:, ic * 128:(ic + 1) * 128],
                                     dens[:, hb, :], start=True, stop=True)
                    nc.vector.tensor_tensor(ctx_sb[:, hb, csl], ctx_sb[:, hb, csl],
                                            rp[:, :c.IC], OP.mult)
                # out-proj (+ b_out/8) after the batch's attention
                fco = min(c.FC, c.NPB)
                for mt in range(c.DOT):
                    for fo in range(c.NPB // fco):
                        yp = yps.tile([128, fco], F32, tag="sp", name="yp")
                        nc.tensor.matmul(yp[:], wout[:, mt * 128:(mt + 1) * 128],
                                         ctx_sb[:, hb, fo * fco:(fo + 1) * fco],
                                         start=True, stop=True)
                        yt = ysb.tile([128, fco], F32, tag="yt", name="yt")
                        if (mt + fo) % 2 == 0:
                            nc.scalar.activation(yt[:], yp[:], AF.Identity,
                                                 bias=bout[:, mt:mt + 1])
                        else:
                            nc.vector.tensor_scalar_add(yt[:], yp[:], bout[:, mt:mt + 1])
                        nc.sync.dma_start(
                            out_d[:, mt, hoff + fo * fco: hoff + (fo + 1) * fco], yt[:])

    if split_waits:
        split_excess_waits(nc)
    nc.finalize()
    return nc


# ---------------------------------------------------------------------------
# host side
def host_inputs(c: Cfg, core: int, x, ln_w, ln_b, w_qkv, w_out, b_out):
    """Build the per-core input dict (all numpy, layouts described in build_nc)."""
    DIM, DH, H = c.DIM, c.DH, c.H
    INNER = w_qkv.shape[1] // 3
    TOK = c.TOK
    xf = x.reshape(TOK, DIM)
    xt = np.ascontiguousarray(xf.T.reshape(c.KC, 128, TOK).transpose(1, 0, 2))

    cs = core * c.QC
    sc = DH ** -0.5
    wsl = np.concatenate([w_qkv[:, cs:cs + c.QC] * sc,
                          w_qkv[:, INNER + cs:INNER + cs + c.QC],
                          w_qkv[:, 2 * INNER + cs:2 * INNER + cs + c.QC]], axis=1)
    wq = (ln_w[:, None] * wsl).reshape(c.KC, 128, 3 * c.QC).transpose(1, 0, 2)
    u = ln_w @ wsl   # [384]
    v = ln_b @ wsl
    fixc = np.concatenate([u.reshape(3, c.QC).T, v.reshape(3, c.QC).T], axis=1)

    inv = np.exp(np.arange(0, DH, 2, dtype=np.float64) * (-np.log(10000.0) / DH))
    ang = np.arange(c.NPB, dtype=np.float64)[:, None] * inv[None, :]
    cosR = np.repeat(np.cos(ang), 2, axis=1)
    sinR = np.repeat(np.sin(ang), 2, axis=1)
    sign = np.tile([-1.0, 1.0], DH // 2)
    sinS = sinR * sign[None, :]
    cosk = np.tile(cosR.T, (H, 1))
    sink = np.tile(sinS.T, (H, 1))

    wout = w_out[core * 128:(core + 1) * 128, :]
    bout = b_out.reshape(c.DOT, 128).T

    selstat = np.zeros((c.TFC, c.TFC * 128), np.float32)
    for fc in range(c.TFC):
        selstat[fc, fc * 128:(fc + 1) * 128] = 1.0
    idblk = np.zeros((128, 64), np.float32)
    for i in range(64):
        idblk[i, i] = 1.0
        idblk[64 + i, i] = 1.0
    selden = np.zeros((2 * c.ICN, c.ICN * 128), np.float32)
    for ic in range(c.ICN):
        selden[ic, ic * 128:ic * 128 + 64] = 1.0
        selden[c.ICN + ic, ic * 128 + 64:(ic + 1) * 128] = 1.0

    f32 = np.float32
    bf = ml_dtypes.bfloat16
    return {
        "xt": xt.astype(bf), "wq": wq.astype(bf), "fixc": fixc.astype(f32),
        "cosk": cosk.astype(f32), "sink": sink.astype(f32),
        "wout": wout.astype(f32), "bout": bout.astype(f32),
        "selstat": selstat, "selden": selden, "idblk": idblk,
    }


def assemble_output(c: Cfg, outs, B, N):
    yT = np.zeros((c.DO, c.TOK), np.float64)
    for o in outs:
        yT += o.transpose(1, 0, 2).reshape(c.DO, c.TOK).astype(np.float64)
    return np.ascontiguousarray(yT.T.reshape(B, N, c.DIM)).astype(np.float32)


_NC_CACHE = {}


def kernel(x, ln_w, ln_b, w_qkv, w_out, b_out):
    from concourse.bass_utils import run_bass_kernel_spmd

    x = np.asarray(x, np.float32)
    ln_w = np.asarray(ln_w, np.float32)
    ln_b = np.asarray(ln_b, np.float32)
    w_qkv = np.asarray(w_qkv, np.float32)
    w_out = np.asarray(w_out, np.float32)
    b_out = np.asarray(b_out, np.float32)

    B, N, DIM = x.shape
    c = Cfg(DIM=DIM, NB=B, NPB=N)
    key = (DIM, B, N)
    if key not in _NC_CACHE:
        _NC_CACHE[key] = build_nc(c)
    nc = _NC_CACHE[key]
    in_maps = [host_inputs(c, core, x, ln_w, ln_b, w_qkv, w_out, b_out)
               for core in range(8)]
    import time as _time
    last = None
    for attempt in range(3):
        try:
            res = run_bass_kernel_spmd(nc, in_maps, core_ids=list(range(8)))
            break
        except Exception as e:  # transient device-unrecoverable wedges recover on retry
            last = e
            _time.sleep(15)
    else:
        raise last
    return assemble_output(c, [res.results[cc]["out"] for cc in range(8)], B, N)

